# revision 36
# baseline (speedup 1.0000x reference)
"""BiLSTM-CRF forward (NLL) on 8 NeuronCores via Bass/Tile.

Sharding: batch (64) is split 8 ways; core q owns sequences [8q, 8q+8) and
runs BOTH LSTM directions locally (fwd chain F + bwd chain B on host-time-
reversed tokens, staggered by one step), so the summed emissions never leave
the core — no collective. The CRF runs locally too: an exp-domain alpha
stream (t in [0,128)) and beta stream (t in (128,256], on reversed
emissions with trans^T) meet at t=127/128; the meet-point states and
gold-score partial sums are combined on the host.

Cell elementwise uses tanh(g) = 2*sigmoid(2g) - 1 with the factor 2 folded
into the g-gate weights host-side, so one Sigmoid activation covers all
four gates; work is spread across DVE and GpSimd to shorten the per-step
dependency chain.

Numerics: embeddings/weights bf16 (fp32 PSUM accumulation), LSTM cell
state fp32, CRF in the exp domain with a per-step shift DELTA=log(17).
"""
import sys
import os

sys.path.insert(0, '/opt/trn_rl_repo')

import numpy as np
import ml_dtypes

import concourse.bass as bass
import concourse.mybir as mybir
import concourse.tile as tile
from concourse import bass_utils
import bass_rust

F32 = mybir.dt.float32
BF16 = mybir.dt.bfloat16
I32 = mybir.dt.int32
AF = mybir.ActivationFunctionType
ALU = mybir.AluOpType

B, T, E, H2, K = 64, 256, 256, 256, 17
G = 4 * H2            # 1024 gates per direction
BQ = 8                # sequences per core
NC_CHUNKS = 32
S = T // NC_CHUNKS    # 8 steps per chunk
HALF = T // 2         # 128
DELTA = float(np.log(K))
NB = HALF * BQ        # 1024 cols per CRF stream
NT = (HALF + 1) * BQ  # 1032 cols per gold-tag stream

_ctr = [0]


def _legalize(nc):
    """Split multi-wait instructions (this walrus supports 1 wait/inst)
    into same-engine NoOp(wait) chains; drop unencodable SeqAsserts."""
    for f in nc.m.functions:
        for blk in f.blocks:
            out = []
            changed = False
            for ins in blk.instructions:
                if ins.opcode == "ISA" and getattr(ins, "op_name", "") == "SeqAssert":
                    si = ins.sync_info
                    if si is not None and (si.on_wait or si.on_update):
                        _ctr[0] += 1
                        nop = bass_rust.InstNoOp(name=f"anop_{_ctr[0]}",
                                                 engine=ins.engine, ins=[], outs=[])
                        nop.sync_info = si
                        out.append(nop)
                    changed = True
                    continue
                si = ins.sync_info
                if si is not None and si.on_wait is not None and len(si.on_wait) > 1:
                    waits = list(si.on_wait)
                    for w in waits[:-1]:
                        _ctr[0] += 1
                        nop = bass_rust.InstNoOp(name=f"wnop_{_ctr[0]}",
                                                 engine=ins.engine, ins=[], outs=[])
                        nop.sync_info = mybir.SyncInfo(on_wait=[w], on_update=[])
                        out.append(nop)
                    ins.sync_info = mybir.SyncInfo(on_wait=[waits[-1]],
                                                  on_update=list(si.on_update or []))
                    changed = True
                out.append(ins)
            if changed:
                blk.instructions = out
    return nc


def build_nc():
    nc = bass.Bass(num_devices=8, enable_asserts=False)

    # ---------------- I/O ----------------
    emb = nc.dram_tensor("emb", [50000, E], BF16, kind="ExternalInput")
    idx = nc.dram_tensor("idx", [128, 32], I32, kind="ExternalInput")
    wih_f = nc.dram_tensor("wih_f", [E, G], BF16, kind="ExternalInput")
    wih_b = nc.dram_tensor("wih_b", [E, G], BF16, kind="ExternalInput")
    whh_f = nc.dram_tensor("whh_f", [H2, G], BF16, kind="ExternalInput")
    whh_b = nc.dram_tensor("whh_b", [H2, G], BF16, kind="ExternalInput")
    gb_f = nc.dram_tensor("gb_f", [8, 128], BF16, kind="ExternalInput")
    gb_b = nc.dram_tensor("gb_b", [8, 128], BF16, kind="ExternalInput")
    ind8 = nc.dram_tensor("ind8", [8, 512], BF16, kind="ExternalInput")
    wout_f = nc.dram_tensor("wout_f", [128, 2 * K], BF16, kind="ExternalInput")
    wout_b = nc.dram_tensor("wout_b", [128, 2 * K], BF16, kind="ExternalInput")
    trans_a = nc.dram_tensor("trans_a", [K, K], F32, kind="ExternalInput")
    trans_b = nc.dram_tensor("trans_b", [K, K], F32, kind="ExternalInput")
    sv = nc.dram_tensor("sv", [K, 1], F32, kind="ExternalInput")
    ev = nc.dram_tensor("ev", [K, 1], F32, kind="ExternalInput")
    bmd = nc.dram_tensor("bmd", [K, 1], F32, kind="ExternalInput")
    m_ord = nc.dram_tensor("m_ord", [1, 2 * NB], F32, kind="ExternalInput")

    o_va = nc.dram_tensor("o_va", [K, BQ], F32, kind="ExternalOutput")
    o_vb = nc.dram_tensor("o_vb", [K, BQ], F32, kind="ExternalOutput")
    o_e = nc.dram_tensor("o_e", [K, T * BQ], F32, kind="ExternalOutput")

    with tile.TileContext(nc) as tc:
        with tc.tile_pool(name="const", bufs=1) as cp:
            # ------------- constant loads (gather index first: the token
            # gather pipeline gates kernel start, the weights do not) -------
            idx_sb = cp.tile([128, 32], I32)
            nc.sync.dma_start(idx_sb[:], idx[:])
            wih_sb = {}
            whh_sb = {}
            wout_sb = {}
            gb_sb = {}
            for ch, (wi, wh, wo, gb) in enumerate(
                    [(wih_f, whh_f, wout_f, gb_f), (wih_b, whh_b, wout_b, gb_b)]):
                wih_sb[ch] = cp.tile([128, 2 * G], BF16, name=f"wih{ch}")
                nc.sync.dma_start(wih_sb[ch][:, 0:G], wi[0:128, :])
                nc.sync.dma_start(wih_sb[ch][:, G:2 * G], wi[128:256, :])
                whh_sb[ch] = cp.tile([128, 2 * G], BF16, name=f"whh{ch}")
                nc.sync.dma_start(whh_sb[ch][:, 0:G], wh[0:128, :])
                nc.sync.dma_start(whh_sb[ch][:, G:2 * G], wh[128:256, :])
                wout_sb[ch] = cp.tile([128, 2 * K], BF16, name=f"wout{ch}")
                nc.sync.dma_start(wout_sb[ch][:], wo[:])
                gb_sb[ch] = cp.tile([8, 128], BF16, name=f"gb{ch}")
                nc.sync.dma_start(gb_sb[ch][:], gb[:])
            ind_sb = cp.tile([8, 512], BF16)
            nc.sync.dma_start(ind_sb[:], ind8[:])
            tra_sb = cp.tile([K, K], F32)
            nc.sync.dma_start(tra_sb[:], trans_a[:])
            trb_sb = cp.tile([K, K], F32)
            nc.sync.dma_start(trb_sb[:], trans_b[:])
            sv_sb = cp.tile([K, 1], F32)
            nc.sync.dma_start(sv_sb[:], sv[:])
            ev_sb = cp.tile([K, 1], F32)
            nc.sync.dma_start(ev_sb[:], ev[:])
            bmd_sb = cp.tile([K, 1], F32)
            nc.sync.dma_start(bmd_sb[:], bmd[:])
            ev64 = cp.tile([64, 1], F32)
            nc.sync.dma_start(ev64[32:32 + K, :], ev[:])

            # [exp(trans); 0; ident] augmented stationary operands for the CRF:
            # rows 0:17 = exp(trans), rows 32:49 = identity (fbb pass-through)
            iot64 = cp.tile([64, 1], I32)
            nc.gpsimd.iota(iot64[:], pattern=[[0, 1]], base=0, channel_multiplier=1)
            iotf64 = cp.tile([64, 1], F32)
            nc.vector.tensor_copy(iotf64[:], iot64[:])
            nc.vector.tensor_scalar_add(iotf64[:], iotf64[:], -32.0)
            rowi64 = cp.tile([64, K], I32)
            nc.gpsimd.iota(rowi64[:], pattern=[[1, K]], base=0, channel_multiplier=0)
            lhs_aug = [cp.tile([64, K], F32, name=f"lhsaug{st}") for st in range(2)]
            for st in range(2):
                rf = cp.tile([64, K], F32, name=f"rf{st}")
                nc.vector.tensor_copy(rf[:], rowi64[:])
                nc.vector.tensor_scalar(out=lhs_aug[st][:], in0=rf[:],
                                        scalar1=iotf64[:, 0:1],
                                        scalar2=None, op0=ALU.is_equal)
                nc.scalar.activation(lhs_aug[st][0:K, :],
                                     [tra_sb, trb_sb][st][:], AF.Exp)
            # u buffer: rows 0:17 per-step u = v*cb, rows 32:49 fbb; the
            # dead rows must be zero for the 49-partition contraction
            ubuf = cp.tile([64, 2 * NB], F32)
            nc.vector.memset(ubuf[:], 0.0)

            # e-independent CRF prep, done here so it overlaps the LSTM:
            # mask broadcasts, fbb = (1-m)*exp(end) at rows 32:49 (alpha is
            # never masked for len >= 128; beta resets to its init exp(end)),
            # and the exp-domain v inits
            m64 = cp.tile([64, 2 * NB], F32)
            nc.sync.dma_start(m64[0:K, :], m_ord[:].to_broadcast([K, 2 * NB]))
            nc.sync.dma_start(m64[32:32 + K, :],
                              m_ord[:].to_broadcast([K, 2 * NB]))
            qe_v = cp.tile([K, 1], F32)
            nc.scalar.activation(qe_v[:], ev_sb[:], AF.Exp)
            qs_v = cp.tile([K, 1], F32)
            nc.scalar.activation(qs_v[:], sv_sb[:], AF.Exp)
            qe64 = cp.tile([64, 1], F32)
            nc.scalar.activation(qe64[32:32 + K, :], ev64[32:32 + K, :], AF.Exp)
            nc.vector.tensor_scalar(out=ubuf[32:32 + K, :],
                                    in0=m64[32:32 + K, :],
                                    scalar1=-1.0, scalar2=1.0,
                                    op0=ALU.mult, op1=ALU.add)
            nc.vector.tensor_scalar_mul(ubuf[32:32 + K, :],
                                        ubuf[32:32 + K, :],
                                        qe64[32:32 + K, 0:1])
            v_sb = cp.tile([K, 2 * BQ], F32)
            nc.vector.tensor_copy(v_sb[:, 0:BQ], qs_v[:, 0:1].to_broadcast([K, BQ]))
            nc.vector.tensor_copy(v_sb[:, BQ:2 * BQ],
                                  qe_v[:, 0:1].to_broadcast([K, BQ]))

            xT = cp.tile([128, 2 * 4096], BF16)     # [p, kc*4096 + n]
            hbuf = [cp.tile([128, 2 * 2048], BF16, name=f"hbuf{ch}") for ch in range(2)]
            # per-chain emissions in scan order (F: t asc, B: t desc)
            e_ch = [cp.tile([K, T * BQ], F32, name=f"e{ch}") for ch in range(2)]
            hz = cp.tile([128, 16], BF16)
            nc.vector.memset(hz[:], 0.0)

            # --------- phase 1: gather/transpose + staggered bidi LSTM ---------
            with tc.tile_pool(name="gat", bufs=4) as gatp, \
                 tc.tile_pool(name="lstm_ps", bufs=2, space="PSUM") as pgp, \
                 tc.tile_pool(name="em_ps", bufs=2, space="PSUM") as pep, \
                 tc.tile_pool(name="lstm_sb", bufs=3) as lsb, \
                 tc.tile_pool(name="state", bufs=1) as stp:

                def fetch(j):
                    # gather tile j = chunk j tokens (both chains), token
                    # n = 128*j + p with p = ch*64 + sl*8 + b
                    g_bf = gatp.tile([128, E], BF16, tag="g")
                    nc.gpsimd.indirect_dma_start(
                        out=g_bf[:], out_offset=None, in_=emb[:],
                        in_offset=bass.IndirectOffsetOnAxis(ap=idx_sb[:, j:j + 1], axis=0))
                    nc.sync.dma_start_transpose(
                        xT[:, 128 * j: 128 * (j + 1)], g_bf[:, 0:128])
                    nc.scalar.dma_start_transpose(
                        xT[:, 4096 + 128 * j: 4096 + 128 * (j + 1)], g_bf[:, 128:256])

                PF = 4
                for j in range(PF):
                    fetch(j)

                c_st = [stp.tile([128, 16], F32, name=f"c{ch}") for ch in range(2)]
                for ch in range(2):
                    nc.vector.memset(c_st[ch][:], 0.0)

                pg_tiles = {}
                pe_tiles = {}

                def open_chunk(cc):
                    """Allocate chunk tiles; return deferred input-MM emitters
                    (spread across iterations to keep the PE queue smooth)."""
                    if cc >= NC_CHUNKS:
                        return []
                    if cc + PF < NC_CHUNKS:
                        fetch(cc + PF)
                    ops = []
                    for ch in range(2):
                        pg = pgp.tile([128, 512], F32, tag=f"pg{ch}", name=f"pg{ch}_{cc}")
                        pg_tiles[(cc, ch)] = pg

                        def mk(ch=ch, m=None, pg=pg, cc=cc):
                            for kc in range(2):
                                nc.tensor.matmul(
                                    pg[:, 64 * m: 64 * (m + 1)],
                                    wih_sb[ch][:, G * kc + 128 * m: G * kc + 128 * (m + 1)],
                                    xT[:, 4096 * kc + 128 * cc + 64 * ch:
                                       4096 * kc + 128 * cc + 64 * (ch + 1)],
                                    start=(kc == 0), stop=False)

                        for m in range(8):
                            ops.append(lambda ch=ch, m=m, pg=pg, cc=cc: mk(ch, m, pg, cc))

                        def mkb(ch=ch, pg=pg):
                            nc.tensor.matmul(pg[:, 0:512], gb_sb[ch][:], ind_sb[:],
                                             start=False, stop=False,
                                             skip_group_check=True)
                        ops.append(mkb)
                    return ops

                def chain_step(ch, r):
                    """Recurrence matmuls + cell elementwise for chain ch, step r."""
                    cc, sl = r // S, r % S
                    pg = pg_tiles[(cc, ch)]
                    if r == 0:
                        h_rhs = [hz[:, 0:8], hz[:, 8:16]]
                    else:
                        h_rhs = [hbuf[ch][:, 2048 * kc + 8 * (r - 1): 2048 * kc + 8 * r]
                                 for kc in range(2)]
                    for m in range(8):
                        for kc in range(2):
                            nc.tensor.matmul(
                                pg[:, 64 * m + 8 * sl: 64 * m + 8 * (sl + 1)],
                                whh_sb[ch][:, G * kc + 128 * m: G * kc + 128 * (m + 1)],
                                h_rhs[kc],
                                start=False, stop=(m == 7 and kc == 1))
                    # gates: m0,1=i  m2,3=f  m4,5=g' (pre-scaled 2x)  m6,7=o
                    gsb = lsb.tile([128, 64], F32, tag=f"gs{ch}")
                    pgv = pg[:].rearrange("p (m sl b) -> p sl m b", m=8, sl=S)
                    nc.scalar.activation(
                        gsb[:].rearrange("p (m b) -> p m b", m=8),
                        pgv[:, sl], AF.Sigmoid)
                    si, sf, sg, so = (gsb[:, 0:16], gsb[:, 16:32],
                                      gsb[:, 32:48], gsb[:, 48:64])
                    c_s = c_st[ch][:]
                    # c = sf*c + si*(2*sg - 1) = 2*(si*(sg - 0.5)) + sf*c
                    s2 = lsb.tile([128, 16], F32, tag=f"s2{ch}")
                    s2i = nc.vector.scalar_tensor_tensor(
                        out=s2[:], in0=sg, scalar=-0.5, in1=si,
                        op0=ALU.add, op1=ALU.mult)
                    cm = lsb.tile([128, 16], F32, tag=f"cm{ch}")
                    cmi = nc.vector.tensor_mul(cm[:], c_s, sf)
                    ci = nc.vector.scalar_tensor_tensor(
                        out=c_s, in0=s2[:], scalar=2.0, in1=cm[:],
                        op0=ALU.mult, op1=ALU.add)
                    tc_t = lsb.tile([128, 16], F32, tag=f"tc{ch}")
                    nc.scalar.activation(tc_t[:], c_s, AF.Tanh)
                    return so, tc_t, s2i, cmi, ci

                def chain_h(ch, r, so, tc_t, eng):
                    hv = hbuf[ch][:].rearrange("p (kc u b) -> p kc u b", kc=2, b=8)
                    eng.tensor_mul(
                        hv[:, :, r, :],
                        so.rearrange("p (kc b) -> p kc b", kc=2),
                        tc_t[:].rearrange("p (kc b) -> p kc b", kc=2))

                def close_chunk(cc, ch):
                    """Emissions for chunk cc of chain ch -> own scan-order buffer."""
                    pe_t = pep.tile([K, 64], F32, tag=f"pe{ch}", name=f"pe{ch}_{cc}")
                    for kc in range(2):
                        nc.tensor.matmul(
                            pe_t[:],
                            wout_sb[ch][:, K * kc: K * (kc + 1)],
                            hbuf[ch][:, 2048 * kc + 64 * cc: 2048 * kc + 64 * (cc + 1)],
                            start=(kc == 0), stop=(kc == 1))
                    nc.vector.tensor_copy(
                        e_ch[ch][:, 64 * cc: 64 * (cc + 1)], pe_t[:])

                for op in open_chunk(0):
                    op()
                for r in range(T + 1):
                    tail_ops = []
                    if r % S == 0 and 0 < r < T:
                        ops = open_chunk(r // S)
                        # chain-0 inputs now (needed by this iteration's
                        # F-step); chain-1 inputs at the end of this
                        # iteration (needed from the next iteration's B-step)
                        for op in ops[:9]:
                            op()
                        tail_ops = ops[9:]
                    hF = chain_step(0, r) if r < T else None
                    hB = chain_step(1, r - 1) if r >= 1 else None
                    # pin the static DVE order: BOTH of B's independent trio
                    # ops must follow F's cell update, else the scheduler
                    # hoists one of them into F's critical path where it
                    # stalls the queue waiting on sigma_B
                    if hF is not None and hB is not None:
                        tile.add_dep_helper(
                            hB[2].ins, hF[4].ins, sync=False,
                            reason="B s2 after F cell-update on DVE")
                        tile.add_dep_helper(
                            hB[3].ins, hF[4].ins, sync=False,
                            reason="B cm after F cell-update on DVE")
                    # both h-muls on GpSimd, off the DVE critical queue
                    if hF is not None:
                        chain_h(0, r, hF[0], hF[1], nc.gpsimd)
                    if hB is not None:
                        chain_h(1, r - 1, hB[0], hB[1], nc.gpsimd)
                    for op in tail_ops:
                        op()
                    if r % S == S - 1:
                        close_chunk(r // S, 0)
                    if r % S == 0 and r >= S:
                        close_chunk(r // S - 1, 1)

            # ------------- phase 2: CRF + gold partials -------------
            with tc.tile_pool(name="crf_sb", bufs=1) as csb, \
                 tc.tile_pool(name="crf_ps", bufs=2, space="PSUM") as cps:
                # summed emissions in real-t order: e(t) = eF(t) + eB(255-t)
                e_sb = csb.tile([K, T * BQ], F32)
                ebv = e_ch[1][:].rearrange("p (u b) -> p u b", b=BQ)
                nc.vector.tensor_add(
                    e_sb[:].rearrange("p (u b) -> p u b", b=BQ),
                    e_ch[0][:].rearrange("p (u b) -> p u b", b=BQ),
                    ebv[:, 255::-1, :])
                nc.sync.dma_start(o_e[:], e_sb[:])
                pe_buf = csb.tile([K, 2 * NB], F32)
                nc.scalar.activation(pe_buf[:], e_sb[:], AF.Exp, bias=bmd_sb[:, 0:1])
                # cb per stream in scan order: alpha = t asc, beta = t desc
                cb = csb.tile([K, 2 * NB], F32)
                nc.vector.tensor_mul(cb[:, 0:NB], pe_buf[:, 0:NB], m64[0:K, 0:NB])
                pv = pe_buf[:].rearrange("p (u b) -> p u b", b=BQ)
                nc.vector.tensor_mul(
                    cb[:, NB:2 * NB].rearrange("p (u b) -> p u b", b=BQ),
                    pv[:, 255:127:-1, :],
                    m64[0:K, NB:2 * NB].rearrange("p (u b) -> p u b", b=BQ))

                # ---- exp-domain v recursions: alpha (st 0), beta (st 1) ----
                outa = csb.tile([K, BQ], F32)
                outb = csb.tile([K, BQ], F32)
                vcur = [v_sb[:, 0:BQ], v_sb[:, BQ:2 * BQ]]
                nsteps = [HALF - 1, HALF]
                for s in range(1, HALF + 1):
                    for st in range(2):
                        if s > nsteps[st]:
                            continue
                        c0 = NB * st + BQ * (s - 1)
                        nc.vector.tensor_mul(ubuf[0:K, c0:c0 + BQ], vcur[st],
                                             cb[:, c0:c0 + BQ])
                        vps = cps.tile([K, BQ], F32, tag=f"v{st}")
                        nc.tensor.matmul(vps[:], lhs_aug[st][0:32 + K, :],
                                         ubuf[0:32 + K, c0:c0 + BQ],
                                         start=True, stop=True)
                        if st == 0 and s == HALF - 1:
                            nc.vector.tensor_copy(outa[:], vps[:])
                        if st == 1 and s == HALF:
                            nc.vector.tensor_copy(outb[:], vps[:])
                        vcur[st] = vps[:]
                nc.sync.dma_start(o_va[:], outa[:])
                nc.sync.dma_start(o_vb[:], outb[:])

    return _legalize(nc)


_CACHE = {}


def _get_nc():
    if "nc" not in _CACHE:
        _CACHE["nc"] = build_nc()
    return _CACHE["nc"]


def _prep_dir_weights(w_ih, w_hh, b_ih, b_hh):
    """Scale g-gate rows by 2 (tanh(g) = 2*sigmoid(2g) - 1 trick)."""
    wi = np.array(w_ih, np.float32)
    wh = np.array(w_hh, np.float32)
    gb = (np.asarray(b_ih, np.float32) + np.asarray(b_hh, np.float32)).copy()
    wi[2 * H2:3 * H2] *= 2.0
    wh[2 * H2:3 * H2] *= 2.0
    gb[2 * H2:3 * H2] *= 2.0
    # reorder (i, f, g, o): pytorch order is i, f, g, o already
    wihT = np.ascontiguousarray(wi.T.astype(ml_dtypes.bfloat16))       # [E, G]
    whhT = np.ascontiguousarray(wh.T.astype(ml_dtypes.bfloat16))       # [H2, G]
    gbT = np.ascontiguousarray(gb.reshape(8, 128).astype(ml_dtypes.bfloat16))
    return wihT, whhT, gbT


def _prep_core_inputs(shared, core):
    (emb_bf, wf, wb, woutF, woutB, gbF, gbB, ind8v,
     tags, mask, sent, trans, start, end, b_out) = shared
    bs = slice(BQ * core, BQ * (core + 1))
    sent_q = sent[bs]
    tags_q = tags[bs]
    mask_q = mask[bs].astype(np.float32)

    # token n = cc*128 + ch*64 + sl*8 + b ; idx[p, j]: n = 128*j + p
    t_f = np.arange(T)
    t_b = T - 1 - np.arange(T)
    tok = np.empty((32, 2, 8, 8), np.int32)   # [cc, ch, sl, b]
    for cc in range(32):
        for sl_ in range(8):
            u = cc * 8 + sl_
            tok[cc, 0, sl_, :] = sent_q[:, t_f[u]]
            tok[cc, 1, sl_, :] = sent_q[:, t_b[u]]
    idx = tok.reshape(32, 128).T.copy()       # [p, j]

    uu = np.arange(HALF)
    ma = mask_q[:, uu].T.reshape(-1).astype(np.float32)
    mb = mask_q[:, T - 1 - uu].T.reshape(-1).astype(np.float32)

    return {
        "emb": emb_bf,
        "idx": np.ascontiguousarray(idx),
        "wih_f": wf[0], "whh_f": wf[1], "gb_f": gbF,
        "wih_b": wb[0], "whh_b": wb[1], "gb_b": gbB,
        "ind8": ind8v,
        "wout_f": woutF, "wout_b": woutB,
        "trans_a": np.ascontiguousarray(trans.astype(np.float32)),
        "trans_b": np.ascontiguousarray(trans.T.astype(np.float32)),
        "sv": start.reshape(K, 1).astype(np.float32),
        "ev": end.reshape(K, 1).astype(np.float32),
        "bmd": (b_out - DELTA).reshape(K, 1).astype(np.float32),
        "m_ord": np.ascontiguousarray(np.concatenate([ma, mb]).reshape(1, -1)),
    }


def kernel(sentence, tags, mask, emb, w_ih_f, w_hh_f, b_ih_f, b_hh_f,
           w_ih_b, w_hh_b, b_ih_b, b_hh_b, w_out, b_out,
           start_trans, trans_matrix, end_trans):
    sentence = np.asarray(sentence).astype(np.int64)
    tags = np.asarray(tags).astype(np.int64)
    mask_b = np.asarray(mask).astype(bool)
    emb = np.asarray(emb, np.float32)
    w_out = np.asarray(w_out, np.float32)
    b_out = np.asarray(b_out, np.float32)
    trans = np.asarray(trans_matrix, np.float32)
    start = np.asarray(start_trans, np.float32)
    end = np.asarray(end_trans, np.float32)

    emb_z = emb.copy()
    emb_z[0] = 0.0
    emb_bf = emb_z.astype(ml_dtypes.bfloat16)

    wf = _prep_dir_weights(w_ih_f, w_hh_f, b_ih_f, b_hh_f)
    wb = _prep_dir_weights(w_ih_b, w_hh_b, b_ih_b, b_hh_b)

    def wout_half(d):
        # SBUF layout [128, 2K]: [p, kc*K + k] with weight w_out[k, 256*d+128*kc+p]
        wo = np.empty((128, 2 * K), np.float32)
        for kc in range(2):
            wo[:, K * kc:K * (kc + 1)] = w_out[:, 256 * d + 128 * kc:
                                               256 * d + 128 * (kc + 1)].T
        return np.ascontiguousarray(wo.astype(ml_dtypes.bfloat16))

    ind8v = np.ascontiguousarray(
        (np.arange(512)[None, :] // 64 == np.arange(8)[:, None])
        .astype(ml_dtypes.bfloat16))

    shared = (emb_bf, wf, wb, wout_half(0), wout_half(1),
              wf[2], wb[2], ind8v,
              tags, mask_b, sentence, trans, start, end, b_out)

    nc = _get_nc()
    in_maps = [_prep_core_inputs(shared, c) for c in range(8)]
    res = bass_utils.run_bass_kernel_spmd(nc, in_maps, core_ids=list(range(8)))
    _CACHE["last_results"] = res

    lengths = mask_b.sum(1).astype(np.int64)
    b_out64 = b_out.astype(np.float64)
    trans64 = trans.astype(np.float64)
    loss = 0.0
    for i in range(8):
        r = res.results[i]
        va = np.asarray(r["o_va"], np.float64)
        vb = np.asarray(r["o_vb"], np.float64)
        e_all = np.asarray(r["o_e"], np.float64).reshape(K, T, BQ)
        e127 = e_all[:, 127, :] + b_out64[:, None]
        s_ = (va * np.exp(e127) * vb).sum(0)               # [8]
        Lq = lengths[BQ * i: BQ * (i + 1)]
        logZ = np.log(s_) + (Lq - 1) * DELTA
        # gold path score from the device emissions (host-side)
        tags_q = tags[BQ * i: BQ * (i + 1)]                # [8, T]
        mask_q = mask_b[BQ * i: BQ * (i + 1)].astype(np.float64)
        emit = e_all[tags_q.T, np.arange(T)[:, None], np.arange(BQ)[None, :]]
        emit_score = ((emit + b_out64[tags_q.T]) * mask_q.T).sum(0)
        trans_score = (trans64[tags_q[:, :-1], tags_q[:, 1:]] * mask_q[:, 1:]).sum(1)
        last_idx = np.maximum(Lq - 1, 0)
        last_tags = tags_q[np.arange(BQ), last_idx]
        num = (start.astype(np.float64)[tags_q[:, 0]] + emit_score +
               trans_score + end.astype(np.float64)[last_tags])
        loss += (logZ - num).sum()
    return np.float32(loss)
